# revision 27
# baseline (speedup 1.0000x reference)
"""Trainium2 Bass kernel for nn_Block_57921928954115 (dense transformer block).

Sharding: 8 cores = 4 batches x 2 token-half-shards. Core (b, 0) owns token
chunks {0:256, 768:1024} of batch b; core (b, 1) owns {256:512, 512:768}
(balanced causal load). K/V are computed redundantly within each pair so
attention needs no collectives.

v2 design:
- x arrives host-transposed and fp8-DoubleRow-packed: no device transposes.
- fp8e4 DoubleRow matmuls (2x PE rate) for QKV, Ww, Wg, Wd. bf16 operands
  for scores/PV/Wo/W1/W2 (full PE rate, 2-4x DVE rate, half DMA).
- RMS1 is computed on host (x is a host input): s1 folds into wv, the
  rope-q tables, and the ones-column of the V stationary operand.
- RMS2 needs one scalar AllReduce; s2 is deferred algebraically to the
  Ww/Wg evictions (wx = s2*(raw@Ww) + (b1@Ww+bw)), so ~40us of W1/Ww
  issue-ahead work hides the collective latency. All biases are folded
  into host-precomputed constants (cw, cg, c2).
- exp runs on head PAIRS ([128,512] Act ops; the causal mask is shared
  within a pair); masking is multiplicative-0/1 post-exp, and skipped on
  the 4 kv-tiles that are causal-safe for every core (SPMD).
"""
import numpy as np

B, T, C, H, S = 4, 1024, 768, 12, 64
FF, HID = 3072, 2048
EPS = 1e-6
NC = 8
TOK = 512          # own tokens per core
CHK = 256          # q chunk
NUMEL = float(B * T * C)
CT, FT, HT = C // 128, FF // 128, HID // 128
TT8 = T // 128
SW, SA = 64.0, 16.0          # fp8 weight / activation scales
INV = 1.0 / (SW * SA)

_cache = {}


def _rope_tables_np():
    theta = (10000.0 ** (-2.0 * np.arange(0, S, 2, dtype=np.float32) / S))
    theta = theta.astype(np.float32)
    freqs = np.arange(T, dtype=np.float32)[:, None] * theta[None, :]
    cos = np.repeat(np.cos(freqs), 2, axis=-1).astype(np.float32)  # [T,64]
    sin = np.repeat(np.sin(freqs), 2, axis=-1).astype(np.float32)
    return cos, sin


def _chunks_for(half):
    if half == 0:
        return [(0, 256), (768, 1024)]
    return [(256, 512), (512, 768)]


def _own_idx(half):
    (a0, a1), (b0, b1) = _chunks_for(half)
    return np.concatenate([np.arange(a0, a1), np.arange(b0, b1)])


class _Stop(Exception):
    pass


def _build_program(collectives=True, stop_after=99, reps=1, att_stage=99):
    import concourse.mybir as mybir
    import concourse.tile as tile
    from concourse import bacc

    F32 = mybir.dt.float32
    F32R = mybir.dt.float32r
    BF16 = mybir.dt.bfloat16
    F8 = mybir.dt.float8e4
    AX = mybir.AxisListType.X
    ALU = mybir.AluOpType
    AF = mybir.ActivationFunctionType
    DR = mybir.MatmulPerfMode.DoubleRow

    nc = bacc.Bacc("TRN2", target_bir_lowering=False, debug=False,
                   enable_asserts=True, num_devices=NC)

    def din(name, shape, dt=F32):
        return nc.dram_tensor(name, list(shape), dt, kind="ExternalInput")

    # x paths (host-transposed / packed)
    x8_d = din("x8", [3 * 128, 2 * T], F8)       # DR-packed x^T (k/v rhs)
    xq8_d = din("xq8", [3 * 128, 2 * TOK], F8)   # DR-packed own-x^T (q rhs)
    xqt_d = din("xqt", [C, TOK])                 # own-x^T f32 (residual)
    # weights
    wq_d = din("wq8", [3 * 128, 2 * C], F8)
    wk_d = din("wk8", [3 * 128, 2 * C], F8)
    wv_d = din("wv8", [3 * 128, 2 * C], F8)
    wo_d = din("wob", [C, C], BF16)
    w1_d = din("w1b", [C, FF], BF16)
    ww_d = din("ww8", [4 * 12 * 128, 1024], F8)  # (grp*12+pair) rows
    wg_d = din("wg8", [4 * 12 * 128, 1024], F8)
    wd_d = din("wd8", [6 * 8 * 128, 1024], F8)   # (grp*8+pair) rows
    w2_d = din("w2b", [FF, C], BF16)
    # constants
    cw_d = din("cwc", [128, HT])                 # SA*(b1@Ww+bw) cols
    cg_d = din("cgc", [128, HT])                 # (b1@Wg+bg) cols
    c2_d = din("c2c", [128, CT])                 # (bd@W2+b2) cols
    cosf_d = din("cosfb", [128, T], BF16)
    sinf_d = din("sinfb", [128, T], BF16)
    cosq_d = din("cosqb", [128, TOK], BF16)      # lambda-scaled
    sinq_d = din("sinqb", [128, TOK], BF16)
    rmat_d = din("rmatb", [128, 128], BF16)
    ones_d = din("onesh", [128, H], BF16)        # filled with v0=bf16(1024*s1)
    mlo_d = din("mlo01", [128, 4 * CHK], BF16)   # 0/1 masks per kv tile
    mhi_d = din("mhi01", [128, 4 * CHK], BF16)   # kv tiles 4..7 of hi chunk
    y_d = nc.dram_tensor("yT", [C, TOK], F32, kind="ExternalOutput")

    with tile.TileContext(nc) as tc:
        from contextlib import ExitStack
        for _rep in range(reps):
            es = ExitStack()
            _open = []

            def _new(**kw):
                p = tc.alloc_tile_pool(**kw)
                _open.append(p)
                return p

            def _rel(p):
                p.release()
                _open.remove(p)

            def _cut(n):
                if stop_after <= n:
                    raise _Stop()

            const = es.enter_context(tc.tile_pool(name="const", bufs=1))

            # ---------------- constants ----------------
            # (DMAs for tables/masks are issued AFTER the x/qkv-weight DMAs
            #  below — they are not needed until rope/attention.)
            ones_col = const.tile([128, 1], F32, name="ones_col")
            nc.vector.memset(ones_col[:], 1.0)
            onesh = const.tile([128, H], BF16, name="onesh")
            rmatb = const.tile([128, 128], BF16, name="rmatb")
            cosfb = const.tile([128, T], BF16, name="cosfb")
            sinfb = const.tile([128, T], BF16, name="sinfb")
            cosqb = const.tile([128, TOK], BF16, name="cosqb")
            sinqb = const.tile([128, TOK], BF16, name="sinqb")
            mlo = const.tile([128, 4 * CHK], BF16, name="mlo")
            mhi = const.tile([128, 4 * CHK], BF16, name="mhi")
            cwc = const.tile([128, HT], F32, name="cwc")
            cgc = const.tile([128, HT], F32, name="cgc")
            c2c = const.tile([128, CT], F32, name="c2c")
            scb = const.tile([1, 8], F32, name="scb")      # s2 scalar scratch
            s2w_b = const.tile([128, 1], F32, name="s2w_b")
            s2g_b = const.tile([128, 1], F32, name="s2g_b")
            ss2_sb = const.tile([128, 8], F32, name="ss2_sb")

            dram = es.enter_context(tc.tile_pool(name="dram", bufs=1,
                                                 space="DRAM"))
            ar2_in = dram.tile([1, 1], F32, name="ar2_in")
            ar2_out = dram.tile([1, 1], F32, name="ar2_out")

            try:
                # persistent pools
                p_x = _new(name="p_x", bufs=1)
                p_qk = _new(name="p_qk", bufs=1)
                p_kv = _new(name="p_kv", bufs=1)
                x8 = [p_x.tile([128, 2 * T], F8, name=f"x8_{j}")
                      for j in range(3)]
                xq8 = [p_x.tile([128, 2 * TOK], F8, name=f"xq8_{j}")
                       for j in range(3)]
                xqt = [p_x.tile([128, TOK], F32, name=f"xqt{m}")
                       for m in range(CT)]
                wq8 = [p_x.tile([128, 2 * C], F8, name=f"wq8_{j}")
                       for j in range(3)]
                wk8 = [p_x.tile([128, 2 * C], F8, name=f"wk8_{j}")
                       for j in range(3)]
                wv8 = [p_x.tile([128, 2 * C], F8, name=f"wv8_{j}")
                       for j in range(3)]
                qsb = [p_qk.tile([128, TOK], BF16, name=f"qsb{m}")
                       for m in range(CT)]
                ksb = [p_qk.tile([128, T], BF16, name=f"ksb{m}")
                       for m in range(CT)]
                vsb = [p_kv.tile([128, H * 65], BF16, name=f"vsb{t}")
                       for t in range(TT8)]

                # input DMAs: x + qkv weights first (QKV gate), then consts
                for j in range(3):
                    nc.sync.dma_start(
                        x8[j][:], x8_d.ap()[j * 128:(j + 1) * 128, :])
                    nc.sync.dma_start(
                        xq8[j][:], xq8_d.ap()[j * 128:(j + 1) * 128, :])
                for j in range(3):
                    for t_, d_ in ((wq8, wq_d), (wk8, wk_d), (wv8, wv_d)):
                        nc.sync.dma_start(
                            t_[j][:], d_.ap()[j * 128:(j + 1) * 128, :])
                nc.sync.dma_start(onesh[:], ones_d.ap())
                nc.sync.dma_start(rmatb[:], rmat_d.ap())
                for t_, d_ in ((cosfb, cosf_d), (sinfb, sinf_d),
                               (cosqb, cosq_d), (sinqb, sinq_d),
                               (mlo, mlo_d), (mhi, mhi_d)):
                    nc.sync.dma_start(t_[:], d_.ap())
                for m in range(CT):
                    nc.sync.dma_start(
                        xqt[m][:], xqt_d.ap()[m * 128:(m + 1) * 128, :])
                for t_, d_ in ((cwc, cw_d), (cgc, cg_d), (c2c, c2_d)):
                    nc.sync.dma_start(t_[:], d_.ap())

                def re2(t, n):
                    return t[:].rearrange("p (two n) -> p two n", two=2)

                # ---------------- QKV (fp8 DoubleRow) ----------------
                mmp = _new(name="mmp", bufs=6, space="PSUM", side="right")
                # q: [feat m, own tok]; per (m, ci) psum [128,256]
                for m in range(CT):
                    for ci in range(2):
                        qp = mmp.tile([128, CHK], F32, name="qp", tag="mm")
                        for j in range(3):
                            nc.tensor.matmul(
                                qp[:],
                                lhsT=re2(wq8[j], C)[:, :,
                                                    m * 128:(m + 1) * 128],
                                rhs=re2(xq8[j], TOK)[:, :,
                                                     ci * CHK:(ci + 1) * CHK],
                                start=(j == 0), stop=(j == 2), perf_mode=DR)
                        nc.scalar.copy(qsb[m][:, ci * CHK:(ci + 1) * CHK],
                                       qp[:])
                # k: [feat m, tok]; per (m, nn) psum [128,512]
                for m in range(CT):
                    for nn in range(2):
                        kp = mmp.tile([128, 512], F32, name="kp", tag="mm")
                        for j in range(3):
                            nc.tensor.matmul(
                                kp[:],
                                lhsT=re2(wk8[j], C)[:, :,
                                                    m * 128:(m + 1) * 128],
                                rhs=re2(x8[j], T)[:, :,
                                                  nn * 512:(nn + 1) * 512],
                                start=(j == 0), stop=(j == 2), perf_mode=DR)
                        nc.vector.tensor_copy(
                            ksb[m][:, nn * 512:(nn + 1) * 512], kp[:])
                # v: [tok tt, feat]; per (tt, nn) psum [128,384]
                for tt in range(TT8):
                    for nn in range(2):
                        vp = mmp.tile([128, 384], F32, name="vp", tag="mm")
                        for j in range(3):
                            nc.tensor.matmul(
                                vp[:],
                                lhsT=re2(x8[j], T)[:, :,
                                                   tt * 128:(tt + 1) * 128],
                                rhs=re2(wv8[j], C)[:, :,
                                                   nn * 384:(nn + 1) * 384],
                                start=(j == 0), stop=(j == 2), perf_mode=DR)
                        nc.scalar.copy(
                            vsb[tt][:].rearrange("p (h s) -> p h s", s=65)
                            [:, nn * 6:(nn + 1) * 6, 0:64],
                            vp[:].rearrange("p (h s) -> p h s", s=64))
                    nc.vector.tensor_copy(
                        vsb[tt][:].rearrange("p (h s) -> p h s", s=65)
                        [:, :, 64:65].squeeze(),
                        onesh[:])
                _cut(1)

                _rel(mmp)
                _cut(2)

                # -------- rope + attention, interleaved per head-pair -------
                p_out = _new(name="p_out", bufs=1, side="right")
                outT = [p_out.tile([128, TOK], F32, name=f"outT{m}")
                        for m in range(CT)]
                outTr = [p_out.tile([128, TOK], BF16, name=f"outTr{m}")
                         for m in range(CT)]
                p_at = _new(name="p_at", bufs=1, side="right")
                attnT = [p_at.tile([128, TOK], BF16, name=f"attnT{m}")
                         for m in range(CT)]
                with tc.tile_pool(name="attsb", bufs=1) as asb, \
                     tc.tile_pool(name="ropet", bufs=4) as rtmp, \
                     tc.tile_pool(name="scps", bufs=4, space="PSUM") as scps, \
                     tc.tile_pool(name="atps", bufs=2, space="PSUM") as atps:

                    def rope(tiles, m, cos_t, sin_t, ntok):
                        for nn in range(ntok // 512):
                            sl = slice(nn * 512, (nn + 1) * 512)
                            rp = scps.tile([128, 512], F32, name="rp",
                                           tag="rp", bufs=2)
                            nc.tensor.matmul(rp[:], lhsT=rmatb[:],
                                             rhs=tiles[m][:, sl],
                                             start=True, stop=True)
                            ptb = rtmp.tile([128, 512], BF16, name="ptb",
                                            tag="ptb", bufs=3)
                            nc.vector.tensor_copy(ptb[:], rp[:])
                            t1 = rtmp.tile([128, 512], BF16, name="rt1",
                                           tag="rt", bufs=4)
                            nc.vector.tensor_tensor(
                                t1[:], ptb[:], sin_t[:, sl], op=ALU.mult)
                            t2 = rtmp.tile([128, 512], BF16, name="rt2",
                                           tag="rt", bufs=4)
                            nc.vector.tensor_tensor(
                                t2[:], tiles[m][:, sl], cos_t[:, sl],
                                op=ALU.mult)
                            nc.vector.tensor_tensor(
                                tiles[m][:, sl], t1[:], t2[:], op=ALU.add)

                    for mt in range(CT):
                        rope(ksb, mt, cosfb, sinfb, T)
                        rope(qsb, mt, cosqb, sinqb, TOK)
                        for ci, (ktn, mask) in enumerate(((4, mlo), (8, mhi))):
                            qsl = slice(ci * CHK, (ci + 1) * CHK)
                            prs = {}
                            for kt in range(ktn):
                                for hh in range(2):
                                    po = hh * 64
                                    sp = scps.tile([128, CHK], F32, name="sp",
                                                   tag="sc", bufs=4)
                                    nc.tensor.matmul(
                                        sp[:],
                                        lhsT=ksb[mt][po:po + 64,
                                                     kt * 128:(kt + 1) * 128],
                                        rhs=qsb[mt][po:po + 64, qsl],
                                        start=True, stop=True)
                                    pr = asb.tile([128, CHK], BF16, name="pr",
                                                  tag="pr", bufs=18)
                                    prs[(kt, hh)] = pr
                                    if att_stage < 1:
                                        continue
                                    if ci == 1 and kt < 4:
                                        # causal-safe for every core: no mask
                                        nc.scalar.activation(pr[:], sp[:],
                                                             AF.Exp)
                                    elif att_stage < 2:
                                        nc.scalar.activation(pr[:], sp[:],
                                                             AF.Exp)
                                    else:
                                        pr0 = asb.tile([128, CHK], BF16,
                                                       name="pr0", tag="pr0",
                                                       bufs=4)
                                        nc.scalar.activation(pr0[:], sp[:],
                                                             AF.Exp)
                                        mi_ = kt if ci == 0 else kt - 4
                                        nc.vector.tensor_tensor(
                                            pr[:], pr0[:],
                                            mask[:,
                                                 mi_ * CHK:(mi_ + 1) * CHK],
                                            op=ALU.mult)
                            if att_stage < 3:
                                continue
                            for hh in range(2):
                                h = 2 * mt + hh
                                po = hh * 64
                                ap = atps.tile([65, CHK], F32, name="ap",
                                               tag="at")
                                for kt in range(ktn):
                                    nc.tensor.matmul(
                                        ap[:],
                                        lhsT=vsb[kt][:, h * 65:(h + 1) * 65],
                                        rhs=prs[(kt, hh)][:],
                                        start=(kt == 0), stop=(kt == ktn - 1))
                                if att_stage < 4:
                                    continue
                                rcp = asb.tile([1, CHK], F32, name="rcp",
                                               tag="rcp", bufs=3)
                                nc.vector.reciprocal(rcp[:], ap[64:65, :])
                                if att_stage < 5:
                                    continue
                                rcb = asb.tile([64, CHK], F32, name="rcb",
                                               tag="rcb", bufs=3)
                                nc.gpsimd.partition_broadcast(rcb[:], rcp[:])
                                nc.vector.tensor_tensor(
                                    attnT[mt][po:po + 64, qsl],
                                    ap[0:64, :], rcb[:], op=ALU.mult)

                _rel(p_kv)       # free vsb
                _cut(3)

                # ---------------- Wo + residual ----------------
                mmp = _new(name="mmp2", bufs=7, space="PSUM", side="right")
                with tc.tile_pool(name="wop", bufs=3) as wop:
                    for grp in range(2):
                        pts = [mmp.tile([128, 512], F32, name="wops",
                                        tag="mm") for _ in range(3)]
                        for k in range(CT):
                            wt = wop.tile([128, 384], BF16, name="wot",
                                          tag="wot")
                            nc.sync.dma_start(
                                wt[:], wo_d.ap()[k * 128:(k + 1) * 128,
                                                 grp * 384:(grp + 1) * 384])
                            for mi in range(3):
                                nc.tensor.matmul(
                                    pts[mi][:],
                                    lhsT=wt[:, mi * 128:(mi + 1) * 128],
                                    rhs=attnT[k][:],
                                    start=(k == 0), stop=(k == CT - 1))
                        for mi in range(3):
                            m = grp * 3 + mi
                            nc.vector.tensor_tensor(outT[m][:], pts[mi][:],
                                                    xqt[m][:], op=ALU.add)
                            nc.scalar.copy(outTr[m][:], outT[m][:])

                _rel(p_at)       # free attnT
                _rel(p_qk)       # free q/k

                # ---------------- sum(out^2) -> AR2 ----------------
                s1ps = _new(name="s1ps", bufs=1, space="PSUM")
                with tc.tile_pool(name="sqp2", bufs=2) as sqp:
                    for m in range(CT):
                        sq = sqp.tile([128, TOK], F32, name="sq2", tag="sq")
                        nc.vector.tensor_tensor(sq[:], outT[m][:], outT[m][:],
                                                op=ALU.mult)
                        nc.vector.reduce_sum(ss2_sb[:, m:m + 1], sq[:],
                                             axis=AX)
                nc.vector.tensor_tensor(ss2_sb[:, 6:7], ss2_sb[:, 0:1],
                                        ss2_sb[:, 1:2], op=ALU.add)
                for m in range(2, CT):
                    nc.vector.tensor_tensor(ss2_sb[:, 6:7], ss2_sb[:, 6:7],
                                            ss2_sb[:, m:m + 1], op=ALU.add)
                ssp2 = s1ps.tile([1, 1], F32, name="ssp2", tag="ss")
                nc.tensor.matmul(ssp2[:], lhsT=ss2_sb[:, 6:7],
                                 rhs=ones_col[:], start=True, stop=True)
                nc.vector.tensor_copy(scb[:, 7:8], ssp2[:])
                nc.sync.dma_start(ar2_in[:], scb[:, 7:8])
                _rel(s1ps)
                if collectives:
                    nc.gpsimd.collective_compute(
                        "AllReduce", ALU.add, replica_groups=[list(range(NC))],
                        ins=[ar2_in.opt()], outs=[ar2_out.opt()])
                else:
                    nc.sync.dma_start(ar2_out[:], ar2_in[:])
                _cut(4)

                # ---------------- W1 (bf16) -> uT fp8 pairs ----------------
                _rel(p_x)        # free x8/xq8/xqt/wq8/wk8/wv8
                w1p = _new(name="w1p", bufs=7)
                p_u = _new(name="p_u", bufs=1, side="right")
                uT = [p_u.tile([128, 2 * TOK], F8, name=f"uT{j}")
                      for j in range(FT // 2)]
                for grp in range(6):
                    pts = [mmp.tile([128, 512], F32, name="ups", tag="mm")
                           for _ in range(4)]
                    for k in range(CT):
                        pan = w1p.tile([128, 512], BF16, name="w1pan",
                                       tag="w1pan", bufs=3)
                        nc.sync.dma_start(
                            pan[:], w1_d.ap()[k * 128:(k + 1) * 128,
                                              grp * 512:(grp + 1) * 512])
                        for mi in range(4):
                            nc.tensor.matmul(
                                pts[mi][:],
                                lhsT=pan[:, mi * 128:(mi + 1) * 128],
                                rhs=outTr[k][:],
                                start=(k == 0), stop=(k == CT - 1))
                    for mi in range(4):
                        m = grp * 4 + mi
                        nc.scalar.activation(
                            uT[m // 2][:, (m % 2) * TOK:(m % 2 + 1) * TOK],
                            pts[mi][:], AF.Identity, scale=SA)
                _rel(w1p)

                # s2 chain (result needed only at wx/vx evictions). Issued
                # AFTER the W1 evictions so no engine queue has work stuck
                # behind the collective wait: the ar2_out read goes through
                # the otherwise-idle Pool DGE queue, and the only ops queued
                # after the sqrt on Act are the Sigmoids, which need s2g
                # anyway.
                nc.gpsimd.dma_start(scb[:, 0:1], ar2_out[:])
                nc.vector.tensor_scalar(scb[:, 1:2], scb[:, 0:1], 1.0 / NUMEL,
                                        EPS, op0=ALU.mult, op1=ALU.add)
                nc.scalar.sqrt(scb[:, 2:3], scb[:, 1:2])
                nc.vector.reciprocal(scb[:, 3:4], scb[:, 2:3])   # s2
                nc.vector.tensor_scalar(scb[:, 4:5], scb[:, 3:4], SA * INV,
                                        None, op0=ALU.mult)      # s2w
                nc.vector.tensor_scalar(scb[:, 5:6], scb[:, 3:4], INV,
                                        None, op0=ALU.mult)      # s2g
                nc.gpsimd.partition_broadcast(s2w_b[:], scb[:, 4:5])
                nc.gpsimd.partition_broadcast(s2g_b[:], scb[:, 5:6])
                _cut(5)

                # ---------------- Ww/Wg (fp8 DR) -> gT fp8 pairs ----------
                wstr = _new(name="wstr", bufs=16)
                p_g = _new(name="p_g", bufs=1)
                gT = [p_g.tile([128, 2 * TOK], F8, name=f"gT{j}")
                      for j in range(HT // 2)]
                with tc.tile_pool(name="wxsb", bufs=1) as wxsb:
                    for grp in range(4):
                        wxs = []
                        pts = [mmp.tile([128, 512], F32, name="wxps",
                                        tag="mm") for _ in range(4)]
                        for j in range(12):
                            wt = wstr.tile([128, 1024], F8, name="wwt",
                                           tag="wst")
                            r0 = (grp * 12 + j) * 128
                            nc.sync.dma_start(wt[:], ww_d.ap()[r0:r0 + 128, :])
                            for mi in range(4):
                                nc.tensor.matmul(
                                    pts[mi][:],
                                    lhsT=re2(wt, 512)[:, :,
                                                      mi * 128:(mi + 1) * 128],
                                    rhs=re2(uT[j], TOK),
                                    start=(j == 0), stop=(j == 11),
                                    perf_mode=DR)
                        for mi in range(4):
                            m = grp * 4 + mi
                            wx = wxsb.tile([128, 512], BF16, name="wxs",
                                           tag="wxs", bufs=5)
                            nc.vector.tensor_scalar(
                                wx[:], pts[mi][:], s2w_b[:], cwc[:, m:m + 1],
                                op0=ALU.mult, op1=ALU.add)
                            wxs.append(wx)
                        pts = [mmp.tile([128, 512], F32, name="vxps",
                                        tag="mm") for _ in range(4)]
                        for j in range(12):
                            wt = wstr.tile([128, 1024], F8, name="wgt",
                                           tag="wst")
                            r0 = (grp * 12 + j) * 128
                            nc.sync.dma_start(wt[:], wg_d.ap()[r0:r0 + 128, :])
                            for mi in range(4):
                                nc.tensor.matmul(
                                    pts[mi][:],
                                    lhsT=re2(wt, 512)[:, :,
                                                      mi * 128:(mi + 1) * 128],
                                    rhs=re2(uT[j], TOK),
                                    start=(j == 0), stop=(j == 11),
                                    perf_mode=DR)
                        for mi in range(4):
                            m = grp * 4 + mi
                            sig = wxsb.tile([128, 512], BF16, name="sig",
                                            tag="sig", bufs=3)
                            nc.scalar.activation(sig[:], pts[mi][:],
                                                 AF.Sigmoid,
                                                 bias=cgc[:, m:m + 1],
                                                 scale=s2g_b[:])
                            vx = wxsb.tile([128, 512], BF16, name="vx",
                                           tag="vx", bufs=3)
                            nc.vector.tensor_scalar(
                                vx[:], pts[mi][:], s2g_b[:], cgc[:, m:m + 1],
                                op0=ALU.mult, op1=ALU.add)
                            vs = wxsb.tile([128, 512], BF16, name="vs",
                                           tag="vs", bufs=3)
                            nc.vector.tensor_tensor(vs[:], vx[:], sig[:],
                                                    op=ALU.mult)
                            nc.vector.tensor_tensor(
                                gT[m // 2][:, (m % 2) * TOK:(m % 2 + 1) * TOK],
                                wxs[mi][:], vs[:], op=ALU.mult)

                _rel(p_u)        # free uT
                _cut(6)

                # ---------------- Wd (fp8 DR) -> ff1 bf16 ----------------
                p_f1 = _new(name="p_f1", bufs=1, side="right")
                ff1b = [p_f1.tile([128, TOK], BF16, name=f"ff1b{m}")
                        for m in range(FT)]
                for grp in range(6):
                    pts = [mmp.tile([128, 512], F32, name="f1ps", tag="mm")
                           for _ in range(4)]
                    for j in range(8):
                        wt = wstr.tile([128, 1024], F8, name="wdt", tag="wst")
                        r0 = (grp * 8 + j) * 128
                        nc.sync.dma_start(wt[:], wd_d.ap()[r0:r0 + 128, :])
                        for mi in range(4):
                            nc.tensor.matmul(
                                pts[mi][:],
                                lhsT=re2(wt, 512)[:, :,
                                                  mi * 128:(mi + 1) * 128],
                                rhs=re2(gT[j], TOK),
                                start=(j == 0), stop=(j == 7), perf_mode=DR)
                    for mi in range(4):
                        m = grp * 4 + mi
                        nc.scalar.activation(ff1b[m][:], pts[mi][:],
                                             AF.Identity, scale=INV)
                _rel(p_g)       # free gT
                _cut(7)

                # ---------------- W2 (bf16) + residual + out ----------------
                # fold c2 into outT ahead of time: 1-op y eviction
                for m in range(CT):
                    nc.vector.tensor_scalar(outT[m][:], outT[m][:],
                                            c2c[:, m:m + 1], None,
                                            op0=ALU.add)
                with tc.tile_pool(name="yout", bufs=3) as yout:
                    for grp in range(2):
                        pts = [mmp.tile([128, 512], F32, name="yps", tag="mm")
                               for _ in range(3)]
                        for k in range(FT):
                            wt = wstr.tile([128, 384], BF16, name="w2t",
                                           tag="wst")
                            nc.sync.dma_start(
                                wt[:], w2_d.ap()[k * 128:(k + 1) * 128,
                                                 grp * 384:(grp + 1) * 384])
                            for mi in range(3):
                                nc.tensor.matmul(
                                    pts[mi][:],
                                    lhsT=wt[:, mi * 128:(mi + 1) * 128],
                                    rhs=ff1b[k][:],
                                    start=(k == 0), stop=(k == FT - 1))
                        for mi in range(3):
                            m = grp * 3 + mi
                            yt = yout.tile([128, 512], F32, name="yt",
                                           tag="yt")
                            nc.vector.tensor_tensor(yt[:], pts[mi][:],
                                                    outT[m][:], op=ALU.add)
                            nc.sync.dma_start(y_d.ap()[m * 128:(m + 1) * 128,
                                                       :], yt[:])

            except _Stop:
                pass
            for _p in reversed(_open):
                _p.release()
            es.close()

    nc.compile()
    return nc


def _drpack(W, scale, e4):
    """[K, N] f32 -> [K/256*128, 2N] fp8, DoubleRow pair-packed."""
    K, N = W.shape
    P = K // 256
    q = (np.asarray(W, np.float32) * scale).astype(e4)
    return np.ascontiguousarray(
        q.reshape(P, 2, 128, N).transpose(0, 2, 1, 3).reshape(P * 128, 2 * N))


def _grppack(Wp, G):
    """DR-packed [P*128, 2N] -> grp-major stream [(G*P)*128, 2*(N/G)]."""
    PT, N2 = Wp.shape
    P = PT // 128
    N = N2 // 2
    cols = N // G
    return np.ascontiguousarray(
        Wp.reshape(P, 128, 2, G, cols).transpose(3, 0, 1, 2, 4)
        .reshape(G * P * 128, 2 * cols))


def _host_inputs(x, Wq, Wk, Wv, Wo, g1, g2, W1, b1, Ww, bw, Wg, bg, Wd, bd,
                 W2, b2):
    import ml_dtypes
    E4 = ml_dtypes.float8_e4m3
    BF = ml_dtypes.bfloat16
    f32 = np.float32
    x = np.asarray(x, f32)
    s1 = f32(1.0 / np.sqrt(EPS + (x.astype(np.float64) ** 2).mean()))

    cos, sin = _rope_tables_np()
    cosf = np.ascontiguousarray(np.tile(cos.T, (2, 1)))     # [128, T]
    sinf = np.ascontiguousarray(np.tile(sin.T, (2, 1)))

    def cols(v, n):
        return np.ascontiguousarray(np.asarray(v, f32).reshape(n, 128).T)

    R = np.zeros((S, S), f32)
    for i in range(S // 2):
        R[2 * i + 1, 2 * i] = -1.0
        R[2 * i, 2 * i + 1] = 1.0
    rmat = np.zeros((128, 128), f32)
    rmat[:64, :64] = R
    rmat[64:, 64:] = R

    g1 = np.asarray(g1, f32)
    g2 = np.asarray(g2, f32)
    wq_f = np.transpose(np.asarray(Wq, f32), (1, 0, 2)).reshape(C, C)
    wk_f = np.transpose(np.asarray(Wk, f32), (1, 0, 2)).reshape(C, C)
    wv_f = np.transpose(np.asarray(Wv, f32), (1, 0, 2)).reshape(C, C)

    # ones-column value and its exact Wo compensation
    v0 = f32(BF(SA * SW * s1))
    gam = (SA * SW * s1) / v0

    lam = (s1 * s1) * (C ** -0.5) * INV * INV   # rope-q table scale

    b1 = np.asarray(b1, f32)
    cw = SA * (b1 @ np.asarray(Ww, f32) + np.asarray(bw, f32))
    cg = b1 @ np.asarray(Wg, f32) + np.asarray(bg, f32)
    c2 = np.asarray(bd, f32) @ np.asarray(W2, f32) + np.asarray(b2, f32)

    shared = {
        "wq8": _drpack(g1[:, None] * wq_f, SW, E4),
        "wk8": _drpack(g1[:, None] * wk_f, SW, E4),
        "wv8": _drpack((g1 * s1)[:, None] * wv_f, SW, E4),
        "wob": np.ascontiguousarray((np.asarray(Wo, f32) / gam).astype(BF)),
        "w1b": np.ascontiguousarray(
            (g2[:, None] * np.asarray(W1, f32)).astype(BF)),
        "ww8": _grppack(_drpack(np.asarray(Ww, f32), SW, E4), 4),
        "wg8": _grppack(_drpack(np.asarray(Wg, f32), SW, E4), 4),
        "wd8": _grppack(_drpack(np.asarray(Wd, f32), SW, E4), 6),
        "w2b": np.ascontiguousarray(np.asarray(W2, f32).astype(BF)),
        "cwc": cols(cw, HT), "cgc": cols(cg, HT), "c2c": cols(c2, CT),
        "cosfb": cosf.astype(BF), "sinfb": sinf.astype(BF),
        "rmatb": rmat.astype(BF),
        "onesh": np.full((128, H), v0, BF),
    }

    in_maps = []
    for core in range(NC):
        b, half = divmod(core, 2)
        idx = _own_idx(half)
        m = dict(shared)
        xT = x[b].T                      # [C, T]
        m["x8"] = _drpack(np.ascontiguousarray(xT), SA, E4)
        m["xq8"] = _drpack(np.ascontiguousarray(xT[:, idx]), SA, E4)
        m["xqt"] = np.ascontiguousarray(xT[:, idx])
        m["cosqb"] = np.ascontiguousarray(cosf[:, idx] * lam).astype(BF)
        m["sinqb"] = np.ascontiguousarray(sinf[:, idx] * lam).astype(BF)
        (l0, l1), (h0, h1) = _chunks_for(half)
        # 0/1 masks: [128 kv-in-tile, kt*CHK + q]
        mlo = np.zeros((128, 4, CHK), f32)
        mhi = np.zeros((128, 4, CHK), f32)
        kvp = np.arange(128)
        for kt in range(4):
            mlo[:, kt, :] = ((kt * 128 + kvp)[:, None]
                             <= np.arange(l0, l1)[None, :])
            mhi[:, kt, :] = (((kt + 4) * 128 + kvp)[:, None]
                             <= np.arange(h0, h1)[None, :])
        m["mlo01"] = mlo.reshape(128, 4 * CHK).astype(BF)
        m["mhi01"] = mhi.reshape(128, 4 * CHK).astype(BF)
        in_maps.append(m)
    return in_maps


def kernel(**inputs):
    from concourse import bass_utils
    if "nc" not in _cache:
        _cache["nc"] = _build_program()
    nc = _cache["nc"]
    in_maps = _host_inputs(**inputs)
    res = bass_utils.run_bass_kernel_spmd(nc, in_maps,
                                          core_ids=list(range(NC)))
    y = np.empty((B, T, C), np.float32)
    for core in range(NC):
        b, half = divmod(core, 2)
        y[b, _own_idx(half), :] = res.results[core]["yT"].T
    return y
